# revision 1
# baseline (speedup 1.0000x reference)
"""Trainium2 Bass kernel for BasicRecurrentEntityEncoder.

Math (per batch b, entity k, step t):
  enc[b,t,:]  = sum_l mask[b,t,l] * emb[prgrph[b,t,l]] * posmask[l,:]
  g           = sigmoid((h+keys)·s) * sent_mask          (mask folded into gate)
  h_tilda     = sigmoid(h@U + keys@V + s@W)
  h           = normalize(h + g*h_tilda)                  (exact when g=0: h is 0 or unit)

Sharding: data-parallel over batch, 8 paragraphs per core.

Per-core on-chip layouts (BL=8 local paragraphs, K=64, D=128 -> 512 state cols):
  feature-major: col c = b*64 + k, tiles [D=128, 512]     (for PE matmuls)
  layout-B:      chunk j = c>>7, partition p = c&127      (for per-(b,k) scalar ops)
                 so b = 2j + (p>>6), k = p&63

Scan step engines: PE does U/V/W matmuls, gate row-dots, transposes;
ACT does sigmoids + psum->sbuf copy; DVE does gate select, the gated update
(scalar_tensor_tensor), squared-norm (tensor_tensor_reduce), and an
rsqrt via int32-domain magic seed + 2 Newton iterations (ACT Rsqrt is banned
and lives in a different activation-table set than Sigmoid anyway).
"""
import numpy as np

import concourse.bass as bass
import concourse.bacc as bacc
import concourse.tile as tile
from concourse import mybir
from concourse.bass_utils import run_bass_kernel_spmd

F32 = mybir.dt.float32
I32 = mybir.dt.int32
AF = mybir.ActivationFunctionType
ALU = mybir.AluOpType

B, T, L, D, K, V = 64, 128, 32, 128, 64, 50000
NCORES = 8
BL = B // NCORES              # 8 paragraphs per core
COLS = BL * K                 # 512 state columns per core
NJ = COLS // 128              # 4 layout-B chunks
WORDS = BL * T * L            # 32768 gathered words per core
CHUNKS = WORDS // 128         # 256
G = 8                         # chunks per gather instruction
NGI = CHUNKS // G             # 32 gather instructions
MAGIC = 0x5F3759DF

_cache = {}

# debug knobs: restrict which phases are built
DBG_PHASE1 = True
DBG_SCAN_T = T
DBG_LVL = 9  # 1: mm+sigmoid; 2: +gate mms+transposes; 3: +gate small ops;
             # 4: +STT hn; 5: +TTR ss; 6: +rsqrt; 7: +apply; 9: full


def _build_nc():
    nc = bacc.Bacc(None, target_bir_lowering=False)

    emb_t = nc.dram_tensor("emb", [V, D], F32, kind="ExternalInput")
    gidx_t = nc.dram_tensor("gidx", [NGI, 128, G], I32, kind="ExternalInput")
    mo_t = nc.dram_tensor("maskones", [NGI, 128, G, 4], F32, kind="ExternalInput")
    posrep_t = nc.dram_tensor("posrep", [128, 128], F32, kind="ExternalInput")
    keysT_t = nc.dram_tensor("keysT", [128, COLS], F32, kind="ExternalInput")
    U_t = nc.dram_tensor("Uw", [D, D], F32, kind="ExternalInput")
    V_t = nc.dram_tensor("Vw", [D, D], F32, kind="ExternalInput")
    W_t = nc.dram_tensor("Ww", [D, D], F32, kind="ExternalInput")
    mscal_t = nc.dram_tensor("maskscal", [128, 4 * T], F32, kind="ExternalInput")
    oh_t = nc.dram_tensor("onehot32", [128, 32], F32, kind="ExternalInput")
    id_t = nc.dram_tensor("ident", [128, 128], F32, kind="ExternalInput")
    out_t = nc.dram_tensor("h_out", [BL, K, D], F32, kind="ExternalOutput")

    with tile.TileContext(nc) as tc:
        with tc.tile_pool(name="persist", bufs=1) as pp:
            posrep = pp.tile([128, 128], F32)
            keysT = pp.tile([128, COLS], F32)
            Uw = pp.tile([D, D], F32)
            Vw = pp.tile([D, D], F32)
            Ww = pp.tile([D, D], F32)
            mscal = pp.tile([128, 4 * T], F32)      # [p, 4t+j] sentence mask
            oh32 = pp.tile([128, 32], F32)
            ident = pp.tile([128, 128], F32)
            encT = pp.tile([128, T * BL], F32)      # [d, t*8+b]
            ksst = pp.tile([128, 4 * T], F32)       # [p, 4t+j]
            nc.sync.dma_start(out=posrep, in_=posrep_t[:, :])
            nc.sync.dma_start(out=keysT, in_=keysT_t[:, :])
            nc.sync.dma_start(out=Uw, in_=U_t[:, :])
            nc.sync.dma_start(out=Vw, in_=V_t[:, :])
            nc.sync.dma_start(out=Ww, in_=W_t[:, :])
            nc.sync.dma_start(out=mscal, in_=mscal_t[:, :])
            nc.sync.dma_start(out=oh32, in_=oh_t[:, :])
            nc.sync.dma_start(out=ident, in_=id_t[:, :])

            # ---------------- Phase 1: gather + sentence encoder ----------
            with tc.tile_pool(name="p1sb", bufs=3) as p1, \
                 tc.tile_pool(name="p1w", bufs=3) as p1w, \
                 tc.tile_pool(name="p1ps", bufs=2, space="PSUM") as p1ps:
                penc = None
                for n in range(NGI if DBG_PHASE1 else 0):
                    idx = p1.tile([128, G], I32, tag="idx")
                    nc.sync.dma_start(out=idx, in_=gidx_t[n, :, :])
                    mo = p1.tile([128, G, 4], F32, tag="mo")
                    nc.sync.dma_start(out=mo, in_=mo_t[n, :, :, :])
                    embg = p1.tile([128, G, 128], F32, tag="embg")
                    for g in range(G):
                        nc.gpsimd.indirect_dma_start(
                            out=embg[:, g, :], out_offset=None, in_=emb_t[:, :],
                            in_offset=bass.IndirectOffsetOnAxis(
                                ap=idx[:, g:g + 1], axis=0))
                    for g in range(G):
                        ch = n * G + g
                        if ch % 32 == 0:
                            penc = p1ps.tile([128, 128], F32, tag="penc")
                        wt = p1w.tile([128, 128], F32, tag="wt")
                        nc.vector.tensor_tensor(
                            out=wt, in0=embg[:, g, :], in1=posrep, op=ALU.mult)
                        nc.tensor.matmul(
                            out=penc[:, (ch % 32) * 4:(ch % 32) * 4 + 4],
                            lhsT=wt, rhs=mo[:, g, :], start=True, stop=True)
                        if ch % 32 == 31:
                            nc.scalar.copy(
                                out=encT[:, (ch // 32) * 128:(ch // 32) * 128 + 128],
                                in_=penc)

            # ---------------- Phase 1.5: ks table -------------------------
            # ks[b,k,t] = sum_d keys[b,k,d]*enc[b,t,d], stored [p, 4t+j]
            with tc.tile_pool(name="ksps", bufs=2, space="PSUM") as ksps:
                for b in range(BL if DBG_PHASE1 else 0):
                    psk = ksps.tile([64, 128], F32, tag="psk")
                    encb = bass.AP(tensor=encT.tensor, offset=encT.offset + b,
                                   ap=[encT.ap[0], [BL, T]])
                    nc.tensor.matmul(out=psk, lhsT=keysT[:, b * 64:(b + 1) * 64],
                                     rhs=encb, start=True, stop=True)
                    nc.vector.tensor_copy(
                        out=ksst[(b & 1) * 64:(b & 1) * 64 + 64, (b >> 1)::4],
                        in_=psk)

            # ---------------- Phase 2: the scan ---------------------------
            with tc.tile_pool(name="st", bufs=2) as stp, \
                 tc.tile_pool(name="sm", bufs=3) as smp, \
                 tc.tile_pool(name="scr", bufs=2) as scrp, \
                 tc.tile_pool(name="psA", bufs=2, space="PSUM") as psA, \
                 tc.tile_pool(name="psB", bufs=2, space="PSUM") as psB, \
                 tc.tile_pool(name="psG", bufs=2, space="PSUM") as psG, \
                 tc.tile_pool(name="psH", bufs=2, space="PSUM") as psH:
                hT = stp.tile([128, COLS], F32, tag="hT")
                hB = stp.tile([128, COLS], F32, tag="hB")
                nc.vector.memset(hT, 0.0)
                nc.vector.memset(hB, 0.0)
                if not DBG_PHASE1:
                    nc.vector.memset(encT, 0.0)
                    nc.vector.memset(ksst, 0.0)

                for t in range(DBG_SCAN_T):
                    s_sl = encT[:, 8 * t:8 * t + 8]
                    # pre-activation: U.T@hT + V.T@keysT + W.T@bcast(s)
                    pA = psA.tile([128, COLS], F32, tag="pA")
                    nc.tensor.matmul(out=pA, lhsT=Uw, rhs=hT,
                                     start=True, stop=False)
                    nc.tensor.matmul(out=pA, lhsT=Vw, rhs=keysT,
                                     start=False, stop=False)
                    s_bc = bass.AP(tensor=encT.tensor,
                                   offset=encT.offset + 8 * t,
                                   ap=[encT.ap[0], [1, BL], [0, K]])
                    nc.tensor.matmul(out=pA, lhsT=Ww, rhs=s_bc,
                                     start=False, stop=True)
                    htT = scrp.tile([128, COLS], F32, tag="htT")
                    nc.scalar.activation(out=htT, in_=pA, func=AF.Sigmoid)
                    if DBG_LVL < 2:
                        continue

                    # gate row-dots: pG[:, 8j+b'] = sum_d hT[d, 128j+p]*s[d,b']
                    pG = psG.tile([128, 32], F32, tag="pG")
                    for j in range(NJ):
                        nc.tensor.matmul(out=pG[:, 8 * j:8 * j + 8],
                                         lhsT=hT[:, 128 * j:128 * (j + 1)],
                                         rhs=s_sl, start=True, stop=True)
                    # transpose h_tilda into layout-B
                    pB = psB.tile([128, COLS], F32, tag="pB")
                    for j in range(NJ):
                        nc.tensor.transpose(out=pB[:, 128 * j:128 * (j + 1)],
                                            in_=htT[:, 128 * j:128 * (j + 1)],
                                            identity=ident)
                    if DBG_LVL < 3:
                        gsc = scrp.tile([128, COLS], F32, tag="gsc")
                        nc.vector.tensor_copy(out=gsc, in_=pB)
                        continue

                    gsel = smp.tile([128, 32], F32, tag="gsel")
                    nc.vector.tensor_tensor(out=gsel, in0=pG, in1=oh32,
                                            op=ALU.mult)
                    graw = smp.tile([128, 4], F32, tag="graw")
                    nc.vector.tensor_reduce(
                        out=graw, in_=gsel.rearrange("p (a b) -> p a b", b=8),
                        axis=mybir.AxisListType.X, op=ALU.add)
                    gks = smp.tile([128, 4], F32, tag="gks")
                    nc.vector.tensor_tensor(out=gks, in0=graw,
                                            in1=ksst[:, 4 * t:4 * t + 4],
                                            op=ALU.add)
                    gs = smp.tile([128, 4], F32, tag="gs")
                    nc.scalar.activation(out=gs, in_=gks, func=AF.Sigmoid)
                    gm = smp.tile([128, 4], F32, tag="gm")
                    nc.vector.tensor_tensor(out=gm, in0=gs,
                                            in1=mscal[:, 4 * t:4 * t + 4],
                                            op=ALU.mult)
                    if DBG_LVL < 4:
                        continue

                    # hn = h + g*h_tilda  (layout B)
                    hnB = scrp.tile([128, COLS], F32, tag="hnB")
                    for j in range(NJ):
                        nc.vector.scalar_tensor_tensor(
                            out=hnB[:, 128 * j:128 * (j + 1)],
                            in0=pB[:, 128 * j:128 * (j + 1)],
                            scalar=gm[:, j:j + 1],
                            in1=hB[:, 128 * j:128 * (j + 1)],
                            op0=ALU.mult, op1=ALU.add)
                    if DBG_LVL < 5:
                        continue
                    # ss = sum_d hn^2  (tensor_tensor_reduce miscomputes on HW;
                    # use square + free-dim reduce instead)
                    ss = smp.tile([128, 4], F32, tag="ss")
                    sq = scrp.tile([128, COLS], F32, tag="sq")
                    nc.vector.tensor_tensor(out=sq, in0=hnB, in1=hnB,
                                            op=ALU.mult)
                    nc.vector.tensor_reduce(
                        out=ss, in_=sq.rearrange("p (a b) -> p a b", b=128),
                        axis=mybir.AxisListType.X, op=ALU.add)
                    if DBG_LVL < 6:
                        continue
                    ssc = smp.tile([128, 4], F32, tag="ssc")
                    nc.vector.tensor_scalar(out=ssc, in0=ss, scalar1=1e-12,
                                            scalar2=None, op0=ALU.max)
                    # inv = rsqrt(ssc): magic seed (int32 value domain) + 2 NR
                    seed = smp.tile([128, 4], I32, tag="seed")
                    nc.vector.tensor_scalar(out=seed, in0=ssc.bitcast(I32),
                                            scalar1=-0.5, scalar2=float(MAGIC),
                                            op0=ALU.mult, op1=ALU.add)
                    y0 = seed.bitcast(F32)
                    t1 = smp.tile([128, 4], F32, tag="t1")
                    t2 = smp.tile([128, 4], F32, tag="t2")
                    t3 = smp.tile([128, 4], F32, tag="t3")
                    y1 = smp.tile([128, 4], F32, tag="y1")
                    nc.vector.tensor_tensor(out=t1, in0=y0, in1=y0, op=ALU.mult)
                    nc.vector.tensor_tensor(out=t2, in0=t1, in1=ssc, op=ALU.mult)
                    nc.vector.tensor_scalar(out=t3, in0=t2, scalar1=-0.5,
                                            scalar2=1.5, op0=ALU.mult, op1=ALU.add)
                    nc.vector.tensor_tensor(out=y1, in0=t3, in1=y0, op=ALU.mult)
                    inv = smp.tile([128, 4], F32, tag="inv")
                    nc.vector.tensor_tensor(out=t1, in0=y1, in1=y1, op=ALU.mult)
                    nc.vector.tensor_tensor(out=t2, in0=t1, in1=ssc, op=ALU.mult)
                    nc.vector.tensor_scalar(out=t3, in0=t2, scalar1=-0.5,
                                            scalar2=1.5, op0=ALU.mult, op1=ALU.add)
                    nc.vector.tensor_tensor(out=inv, in0=t3, in1=y1, op=ALU.mult)

                    if DBG_LVL < 7:
                        continue
                    # h' = hn * inv (layout B), then transpose back
                    hB_new = stp.tile([128, COLS], F32, tag="hB")
                    for j in range(NJ):
                        nc.vector.tensor_scalar(
                            out=hB_new[:, 128 * j:128 * (j + 1)],
                            in0=hnB[:, 128 * j:128 * (j + 1)],
                            scalar1=inv[:, j:j + 1], scalar2=None, op0=ALU.mult)
                    pH = psH.tile([128, COLS], F32, tag="pH")
                    for j in range(NJ):
                        nc.tensor.transpose(out=pH[:, 128 * j:128 * (j + 1)],
                                            in_=hB_new[:, 128 * j:128 * (j + 1)],
                                            identity=ident)
                    hT_new = stp.tile([128, COLS], F32, tag="hT")
                    nc.scalar.copy(out=hT_new, in_=pH)
                    hB, hT = hB_new, hT_new

                # -------- output: h[b,k,:] = hB[(b&1)*64+k, 128*(b>>1)+:] --
                for b in range(BL):
                    src = hB[(b & 1) * 64:(b & 1) * 64 + 64,
                             128 * (b >> 1):128 * (b >> 1) + 128]
                    nc.sync.dma_start(out=out_t[b, :, :], in_=src)
    nc.compile()
    return nc


def _prep_core(core, prgrph, prgrph_mask, embedding_matrix, positional_mask,
               Uw, Vw, Ww, keys):
    b0 = core * BL
    pr = prgrph[b0:b0 + BL]          # [8, T, L]
    pm = prgrph_mask[b0:b0 + BL]
    ky = keys[b0:b0 + BL]            # [8, K, D]

    idx_core = np.ascontiguousarray(pr.transpose(1, 0, 2)).reshape(-1)  # (t,b,l)
    gidx = np.ascontiguousarray(
        idx_core.reshape(NGI, G, 128).transpose(0, 2, 1)).astype(np.int32)

    maskf = pm.transpose(1, 0, 2).reshape(-1).astype(np.float32)
    mw = maskf.reshape(CHUNKS, 4, 32)
    mo = np.zeros((CHUNKS, 128, 4), dtype=np.float32)
    for jj in range(4):
        mo[:, jj * 32:(jj + 1) * 32, jj] = mw[:, jj, :]
    mo = np.ascontiguousarray(
        mo.reshape(NGI, G, 128, 4).transpose(0, 2, 1, 3))

    posrep = np.ascontiguousarray(np.tile(positional_mask, (4, 1))).astype(np.float32)
    keysT = np.ascontiguousarray(ky.transpose(2, 0, 1).reshape(D, COLS))

    # layout-B: partition p, chunk j -> b = 2j + (p>>6)
    p_ar = np.arange(128)
    j_ar = np.arange(4)
    b_of = 2 * j_ar[None, :] + (p_ar[:, None] >> 6)          # [128, 4]
    msent = pm.any(axis=2).astype(np.float32)                # [8, T]
    mscal = np.ascontiguousarray(
        msent[b_of].transpose(0, 2, 1).reshape(128, 4 * T))  # [p, 4t+j]
    oh32 = np.zeros((128, 32), dtype=np.float32)
    for jj in range(4):
        oh32[p_ar, 8 * jj + b_of[:, jj]] = 1.0
    ident = np.eye(128, dtype=np.float32)

    return {
        "emb": np.ascontiguousarray(embedding_matrix.astype(np.float32)),
        "gidx": gidx, "maskones": mo, "posrep": posrep,
        "keysT": keysT,
        "Uw": np.ascontiguousarray(Uw.astype(np.float32)),
        "Vw": np.ascontiguousarray(Vw.astype(np.float32)),
        "Ww": np.ascontiguousarray(Ww.astype(np.float32)),
        "maskscal": mscal, "onehot32": oh32, "ident": ident,
    }


def kernel(prgrph, prgrph_mask, embedding_matrix, positional_mask,
           Uw, Vw, Ww, keys, _trace=False):
    prgrph = np.asarray(prgrph)
    prgrph_mask = np.asarray(prgrph_mask)
    embedding_matrix = np.asarray(embedding_matrix, dtype=np.float32)
    positional_mask = np.asarray(positional_mask, dtype=np.float32)
    Uw = np.asarray(Uw, dtype=np.float32)
    Vw = np.asarray(Vw, dtype=np.float32)
    Ww = np.asarray(Ww, dtype=np.float32)
    keys = np.asarray(keys, dtype=np.float32)

    if "nc" not in _cache:
        _cache["nc"] = _build_nc()
    nc = _cache["nc"]

    in_maps = [_prep_core(c, prgrph, prgrph_mask, embedding_matrix,
                          positional_mask, Uw, Vw, Ww, keys)
               for c in range(NCORES)]
    res = run_bass_kernel_spmd(nc, in_maps, core_ids=list(range(NCORES)),
                               trace=_trace)
    outs = [np.asarray(r["h_out"]).reshape(BL, K, D) for r in res.results]
    full = np.concatenate(outs, axis=0)
    if _trace:
        kernel.last_results = res
    return full

